# revision 32
# baseline (speedup 1.0000x reference)
"""nn_ComplexAttention SPMD Bass kernel for 8 trn2 NeuronCores.

Math (per batch b):
  X = [x_r | x_i]                         [S, 2D]      (S=4096, D=256, 2D=F=512)
  q_flat = X @ Wq_c,  k_flat = X @ Wk_c,  v = X @ Wv_c  (complex linears folded
  into [F, F] block matrices built on the host)
  scores = q_flat @ k_flat.T * D**-0.5 = X @ M @ X.T,   M = Wq_c @ Wk_c.T * D**-0.5
  attn = softmax(scores, axis=-1);  out = [attn @ v_r | attn @ v_i]

Sharding: 8 cores = 4 batches x 2 q-halves (2048 q rows each). Each core
computes, entirely on-device in the [k, q] orientation (k on partitions so
softmax sums and attn @ v contract over partitions via the PE):
  HT = M @ X.T          [F, S]    (fp16, SBUF-resident)
  scoresT = HT.T @ X.T[:, q-half] per 512-col q-quarter, 128-row k-chunks
  expT = exp(scoresT - C) fp16    (C=8.5 keeps exp in fp16 range; cancels)
  Z (partition-sums of expT) via PE matmuls against a ones vector
  attn.T block = expT * (1/Z)     -> HBM  [S, 2048] f32
  out = (expT.T @ v) * (1/Z)      -> HBM  [2048, F] f32
Host side only slices/concats/transposes blocks to full-shape outputs.
"""

import numpy as np

import concourse.bass as bass
import concourse.mybir as mybir
import concourse.tile as tile
from concourse import bacc
from concourse.bass_utils import run_bass_kernel_spmd

B, S, D = 4, 4096, 256
F = 2 * D          # feature dim of the folded complex linear
Q = S // 2         # q rows per core
QW = 512           # q window (quarter of Q)
NQW = Q // QW      # 4 quarters
KC = 128           # k chunk (partition tile)
NKC = S // KC      # 32 k chunks
NFC = F // 128     # 4 feature chunks
EXP_SHIFT = 8.5    # exp(s - C): keeps exp within fp16 range; cancels in softmax
SCALE = D ** -0.5

f32 = mybir.dt.float32
f16 = mybir.dt.float16


def _build_nc():
    nc = bacc.Bacc("TRN2", target_bir_lowering=False, debug=False, num_devices=8)
    xt = nc.dram_tensor("xt", [F, S], f32, kind="ExternalInput").ap()
    m_in = nc.dram_tensor("m_in", [F, F], f16, kind="ExternalInput").ap()
    wv = nc.dram_tensor("wv", [F, F], f16, kind="ExternalInput").ap()
    attn_t = nc.dram_tensor("attn_t", [S, Q], f32, kind="ExternalOutput").ap()
    out = nc.dram_tensor("out", [Q, F], f32, kind="ExternalOutput").ap()
    rz_dram = nc.dram_tensor("rz_scratch", [NQW, QW], f32).ap()

    with tile.TileContext(nc) as tc:
        _emit(tc, xt, m_in, wv, attn_t, out, rz_dram)
    nc.compile()
    return nc


def _emit(tc, xt, m_in, wv, attn_t, out, rz_dram):
    # The host rolls xt's columns so this core's q-half occupies columns
    # 0..Q — one SPMD program serves both halves; attn.T rows come out in
    # the rolled k-order and are un-rolled on the host.
    nc = tc.nc
    KWB = 1024  # xt load/cast column-block

    with (
        tc.tile_pool(name="const", bufs=1) as const,
        tc.tile_pool(name="main", bufs=1) as main,
    ):
        # ---- constants ----
        ones_h = const.tile([128, 1], f16)
        nc.vector.memset(ones_h[:], 1.0)
        bias_c = const.tile([128, 1], f32)
        nc.vector.memset(bias_c[:], -EXP_SHIFT)

        # ---- PE warm-up: dummy matmuls on a zeroed tile during the input
        # DMA window, so the HAM un-throttles (1.2->2.4GHz) before the real
        # matmuls start ----
        warm = const.tile([128, 512], f16)
        nc.vector.memset(warm[:], 0.0)

        m_h = const.tile([128, NFC, F], f16)
        wv_h = const.tile([128, NFC, F], f16)
        xt_h = main.tile([128, NFC, S], f16)
        gt_h = main.tile([128, NFC, Q], f16)
        v_h = main.tile([128, NKC, F], f16)

        with (
            tc.tile_pool(name="psum_prep", bufs=4, space="PSUM") as psum_prep,
            tc.tile_pool(name="stage", bufs=3) as stage_pool,
        ):
            # ---- load weights (already fp16 from the host); m is only
            # needed for GT at kb==1 so it loads after xt's first block ----
            for fi in range(NFC):
                nc.sync.dma_start(wv_h[:, fi], wv[fi * 128:(fi + 1) * 128, :])

            warm_ps = psum_prep.tile([128, 512], f32, tag="warm")
            for _ in range(32):
                nc.tensor.matmul(warm_ps[:], warm[:, 0:128], warm[:],
                                 start=True, stop=True)

            # ---- load + cast X.T in column blocks, computing v (and GT)
            # for each block as soon as its columns land ----
            for kb in range(S // KWB):
                cols = slice(kb * KWB, (kb + 1) * KWB)
                for fi in range(NFC):
                    st = stage_pool.tile([128, KWB], f32, tag="xstage")
                    nc.sync.dma_start(st[:], xt[fi * 128:(fi + 1) * 128, cols])
                    nc.vector.tensor_copy(xt_h[:, fi, cols], st[:])
                if kb == 0:
                    for fi in range(NFC):
                        nc.sync.dma_start(
                            m_h[:, fi], m_in[fi * 128:(fi + 1) * 128, :])

                # v tiles whose 128-col s-window lies in this block
                for sv in range(kb * (KWB // 128), (kb + 1) * (KWB // 128)):
                    ps = psum_prep.tile([128, F], f32, tag="prep")
                    for fi in range(NFC):
                        nc.tensor.matmul(
                            ps[:],
                            xt_h[:, fi, sv * 128:(sv + 1) * 128],
                            wv_h[:, fi],
                            start=(fi == 0), stop=(fi == NFC - 1),
                        )
                    if sv % 2 == 0:
                        nc.vector.tensor_copy(v_h[:, sv], ps[:])
                    else:
                        nc.scalar.copy(v_h[:, sv], ps[:])

                if kb == 1:
                    # q-half columns 0..Q now resident: GT = (Xq @ M).T
                    for fo in range(NFC):
                        for qw in range(Q // 512):
                            ps = psum_prep.tile([128, 512], f32, tag="prep")
                            for fi in range(NFC):
                                nc.tensor.matmul(
                                    ps[:],
                                    m_h[:, fi, fo * 128:(fo + 1) * 128],
                                    xt_h[:, fi, qw * 512:(qw + 1) * 512],
                                    start=(fi == 0), stop=(fi == NFC - 1),
                                )
                            if qw % 2 == 0:
                                nc.vector.tensor_copy(
                                    gt_h[:, fo, qw * 512:(qw + 1) * 512], ps[:])
                            else:
                                nc.scalar.copy(
                                    gt_h[:, fo, qw * 512:(qw + 1) * 512], ps[:])

        # ---- main loop over q quarters ----
        with (
            tc.tile_pool(name="psum_s", bufs=2, space="PSUM") as psum_s,
            tc.tile_pool(name="psum_pv", bufs=4, space="PSUM") as psum_pv,
            tc.tile_pool(name="psum_z", bufs=1, space="PSUM") as psum_z,
            tc.tile_pool(name="expt", bufs=9) as expt_pool,
            tc.tile_pool(name="small", bufs=4) as small,
            tc.tile_pool(name="attn_stage", bufs=4) as attn_stage,
            tc.tile_pool(name="out_stage", bufs=2) as out_stage,
        ):
            for qq in range(NQW):
                qs = qq * QW
                last_q = qq == NQW - 1
                # expt lives as 4 sub-tiles of 8 chunks each so slots free
                # up as their attn-normalize TTs complete (finer-grained
                # than one [128, 32, 512] tile per quarter)
                eg = [expt_pool.tile([128, 8, QW], f16, tag="expt",
                                     name=f"expt_{qq}_{g}")
                      for g in range(NKC // 8)]

                def _ex(c):
                    return eg[c // 8][:, c % 8]

                # -- phase A: scores + exp --
                # For the last quarter, Z is also accumulated here (zrow
                # matmuls, emitted in batches of 4 at least 2 chunks behind
                # the exp producing their input) so the normalize/writeout
                # below can overlap phase B. Earlier quarters get Z from
                # cheap N=1 minis inside phase B instead (their normalize
                # overlaps the NEXT quarter's phase A).
                if last_q:
                    zrow_ps = psum_z.tile([1, QW], f32, tag="zrow",
                                          name=f"zrow_{qq}")

                    def _zrow(zc):
                        nc.tensor.matmul(
                            zrow_ps[:], ones_h[:], _ex(zc),
                            start=(zc == 0), stop=(zc == NKC - 1),
                        )

                z_pending = []
                for c in range(NKC):
                    s_ps = psum_s.tile([128, QW], f32, tag="s")
                    for fi in range(NFC):
                        nc.tensor.matmul(
                            s_ps[:],
                            xt_h[:, fi, c * 128:(c + 1) * 128],
                            gt_h[:, fi, qs:qs + QW],
                            start=(fi == 0), stop=(fi == NFC - 1),
                        )
                    nc.scalar.activation(
                        _ex(c), s_ps[:],
                        mybir.ActivationFunctionType.Exp,
                        bias=bias_c[:], scale=1.0,
                    )
                    if last_q:
                        z_pending.append(c)
                        if len(z_pending) >= 6:
                            for zc in z_pending[:4]:
                                _zrow(zc)
                            z_pending = z_pending[4:]
                if last_q:
                    for zc in z_pending:
                        _zrow(zc)
                    # Z reciprocal + both orientations; overlaps phase B.
                    rz_row = small.tile([1, QW], f32, tag="rzrow")
                    nc.vector.reciprocal(rz_row[:], zrow_ps[:])
                    rz_bc = small.tile([128, QW], f32, tag="rzbc")
                    nc.gpsimd.partition_broadcast(rz_bc[:], rz_row[:])
                    rz_bh = small.tile([128, QW], f16, tag="rzbh")
                    nc.vector.tensor_copy(rz_bh[:], rz_bc[:])
                    nc.sync.dma_start(rz_dram[qq:qq + 1, :], rz_row[:])
                    rzt = small.tile([128, 4], f32, tag="rzt")
                    nc.sync.dma_start(
                        rzt[:], rz_dram[qq].rearrange("(t p) -> p t", p=128))

                    # attn.T writeout emitted here so it overlaps phase B
                    for c in range(NKC):
                        at = attn_stage.tile([128, QW], f32, tag="at")
                        nc.vector.tensor_mul(at[:], _ex(c), rz_bh[:])
                        nc.sync.dma_start(
                            attn_t[c * 128:(c + 1) * 128, qs:qs + QW], at[:])

                # -- phase B: PV accumulation (+ Z minis for qq<3, sharing
                # the already-loaded expt weights; all 4 columns form one
                # accumulation group in one bank) --
                pv_ps = [psum_pv.tile([128, F], f32, tag="pv", name=f"pv_{qq}_{t}")
                         for t in range(4)]
                if not last_q:
                    zt_ps = psum_z.tile([128, 4], f32, tag="zt",
                                        name=f"zt_{qq}")
                for c in range(NKC):
                    for t in range(4):
                        nc.tensor.matmul(
                            pv_ps[t][:], _ex(c)[:, t * 128:(t + 1) * 128],
                            v_h[:, c],
                            start=(c == 0), stop=(c == NKC - 1),
                        )
                        if not last_q:
                            nc.tensor.matmul(
                                zt_ps[:, t:t + 1],
                                _ex(c)[:, t * 128:(t + 1) * 128], ones_h[:],
                                start=(c == 0 and t == 0),
                                stop=(c == NKC - 1 and t == 3),
                            )

                if not last_q:
                    # rzt directly from the partition-oriented Z; row
                    # orientation via the DRAM round-trip (reverse of q3's)
                    rzt = small.tile([128, 4], f32, tag="rzt")
                    nc.vector.reciprocal(rzt[:], zt_ps[:])
                    nc.sync.dma_start(
                        rz_dram[qq].rearrange("(t p) -> p t", p=128), rzt[:])
                    rz_bc = small.tile([128, QW], f32, tag="rzbc")
                    nc.sync.dma_start(
                        rz_bc[:],
                        rz_dram[qq:qq + 1, :].to_broadcast((128, QW)))
                    rz_bh = small.tile([128, QW], f16, tag="rzbh")
                    nc.vector.tensor_copy(rz_bh[:], rz_bc[:])

                    # attn.T writeout; overlaps the next quarter's phase A
                    for c in range(NKC):
                        at = attn_stage.tile([128, QW], f32, tag="at")
                        nc.vector.tensor_mul(at[:], _ex(c), rz_bh[:])
                        nc.sync.dma_start(
                            attn_t[c * 128:(c + 1) * 128, qs:qs + QW], at[:])

                # -- out writeout: pv * (1/Z) (on DVE: keeps ACT's in-order
                # stream free for the next quarter's exp) --
                for t in range(4):
                    o_sb = out_stage.tile([128, F], f32, tag="o")
                    nc.vector.tensor_scalar_mul(o_sb[:], pv_ps[t][:],
                                                rzt[:, t:t + 1])
                    nc.sync.dma_start(
                        out[qs + t * 128:qs + (t + 1) * 128, :], o_sb[:])


def _comb(wr, wi):
    """torch complex nn.Linear y = x @ W.T folded to [x_r|x_i] @ Wc."""
    return np.block([[wr.T, wi.T], [-wi.T, wr.T]])


def _host_prep(inputs):
    f64 = np.float64
    wq = _comb(inputs["wq_real"].astype(f64), inputs["wq_imag"].astype(f64))
    wk = _comb(inputs["wk_real"].astype(f64), inputs["wk_imag"].astype(f64))
    wv = _comb(inputs["wv_real"].astype(f64), inputs["wv_imag"].astype(f64))
    m = ((wq @ wk.T) * SCALE).astype(np.float16)
    wv16 = wv.astype(np.float16)

    in_maps = []
    for c in range(8):
        b, h = c // 2, c % 2
        x = np.concatenate(
            [inputs["query_real"][b], inputs["query_imag"][b]], axis=-1)
        xt = np.ascontiguousarray(x.T.astype(np.float32))
        # roll q columns so this core's q-half occupies columns 0..Q
        if h == 1:
            xt = np.ascontiguousarray(np.roll(xt, -Q, axis=1))
        in_maps.append({"xt": xt, "m_in": m, "wv": wv16})
    return in_maps


_NC = None


def kernel(**inputs):
    global _NC
    if _NC is None:
        _NC = _build_nc()
    in_maps = _host_prep(inputs)
    res = run_bass_kernel_spmd(_NC, in_maps, list(range(8)))

    out_r = np.empty((B, S, D), np.float32)
    out_i = np.empty((B, S, D), np.float32)
    attn = np.empty((B, S, S), np.float32)
    for c in range(8):
        b, h = c // 2, c % 2
        qs = slice(h * Q, (h + 1) * Q)
        o = res.results[c]["out"]
        out_r[b, qs] = o[:, :D]
        out_i[b, qs] = o[:, D:]
        at = res.results[c]["attn_t"]  # [S(k rolled), Q]
        if h == 1:
            at = np.roll(at, Q, axis=0)
        attn[b, qs, :] = at.T
    return out_r, out_i, attn


# revision 33
# speedup vs baseline: 1.0819x; 1.0819x over previous
"""nn_ComplexAttention SPMD Bass kernel for 8 trn2 NeuronCores.

Math (per batch b):
  X = [x_r | x_i]                         [S, 2D]      (S=4096, D=256, 2D=F=512)
  q_flat = X @ Wq_c,  k_flat = X @ Wk_c,  v = X @ Wv_c  (complex linears folded
  into [F, F] block matrices built on the host)
  scores = q_flat @ k_flat.T * D**-0.5 = X @ M @ X.T,   M = Wq_c @ Wk_c.T * D**-0.5
  attn = softmax(scores, axis=-1);  out = [attn @ v_r | attn @ v_i]

Sharding: 8 cores = 4 batches x 2 q-halves (2048 q rows each). Each core
computes, entirely on-device in the [k, q] orientation (k on partitions so
softmax sums and attn @ v contract over partitions via the PE):
  HT = M @ X.T          [F, S]    (fp16, SBUF-resident)
  scoresT = HT.T @ X.T[:, q-half] per 512-col q-quarter, 128-row k-chunks
  expT = exp(scoresT - C) fp16    (C=8.5 keeps exp in fp16 range; cancels)
  Z (partition-sums of expT) via PE matmuls against a ones vector
  attn.T block = expT * (1/Z)     -> HBM  [S, 2048] f32
  out = (expT.T @ v) * (1/Z)      -> HBM  [2048, F] f32
Host side only slices/concats/transposes blocks to full-shape outputs.
"""

import numpy as np

import concourse.bass as bass
import concourse.mybir as mybir
import concourse.tile as tile
from concourse import bacc
from concourse.bass_utils import run_bass_kernel_spmd

B, S, D = 4, 4096, 256
F = 2 * D          # feature dim of the folded complex linear
Q = S // 2         # q rows per core
QW = 512           # q window (quarter of Q)
NQW = Q // QW      # 4 quarters
KC = 128           # k chunk (partition tile)
NKC = S // KC      # 32 k chunks
NFC = F // 128     # 4 feature chunks
EXP_SHIFT = 8.5    # exp(s - C): keeps exp within fp16 range; cancels in softmax
SCALE = D ** -0.5

f32 = mybir.dt.float32
f16 = mybir.dt.float16


def _build_nc():
    nc = bacc.Bacc("TRN2", target_bir_lowering=False, debug=False, num_devices=8)
    xt = nc.dram_tensor("xt", [F, S], f32, kind="ExternalInput").ap()
    m_in = nc.dram_tensor("m_in", [F, F], f16, kind="ExternalInput").ap()
    wv = nc.dram_tensor("wv", [F, F], f16, kind="ExternalInput").ap()
    attn_t = nc.dram_tensor("attn_t", [S, Q], f32, kind="ExternalOutput").ap()
    out = nc.dram_tensor("out", [Q, F], f32, kind="ExternalOutput").ap()
    rz_dram = nc.dram_tensor("rz_scratch", [NQW, QW], f32).ap()

    with tile.TileContext(nc) as tc:
        _emit(tc, xt, m_in, wv, attn_t, out, rz_dram)
    nc.compile()
    return nc


def _emit(tc, xt, m_in, wv, attn_t, out, rz_dram):
    # The host rolls xt's columns so this core's q-half occupies columns
    # 0..Q — one SPMD program serves both halves; attn.T rows come out in
    # the rolled k-order and are un-rolled on the host.
    nc = tc.nc
    KWB = 1024  # xt load/cast column-block

    with (
        tc.tile_pool(name="const", bufs=1) as const,
        tc.tile_pool(name="main", bufs=1) as main,
    ):
        # ---- constants ----
        ones_h = const.tile([128, 1], f16)
        nc.vector.memset(ones_h[:], 1.0)
        bias_c = const.tile([128, 1], f32)
        nc.vector.memset(bias_c[:], -EXP_SHIFT)

        m_h = const.tile([128, NFC, F], f16)
        wv_h = const.tile([128, NFC, F], f16)
        xt_h = main.tile([128, NFC, S], f16)
        gt_h = main.tile([128, NFC, Q], f16)
        v_h = main.tile([128, NKC, F], f16)

        with (
            tc.tile_pool(name="psum_prep", bufs=4, space="PSUM") as psum_prep,
            tc.tile_pool(name="stage", bufs=3) as stage_pool,
        ):
            # ---- load weights (already fp16 from the host); m is only
            # needed for GT at kb==1 so it loads after xt's first block ----
            for fi in range(NFC):
                nc.sync.dma_start(wv_h[:, fi], wv[fi * 128:(fi + 1) * 128, :])

            # ---- load + cast X.T in column blocks, computing v (and GT)
            # for each block as soon as its columns land ----
            for kb in range(S // KWB):
                cols = slice(kb * KWB, (kb + 1) * KWB)
                for fi in range(NFC):
                    st = stage_pool.tile([128, KWB], f32, tag="xstage")
                    nc.sync.dma_start(st[:], xt[fi * 128:(fi + 1) * 128, cols])
                    nc.vector.tensor_copy(xt_h[:, fi, cols], st[:])
                if kb == 0:
                    for fi in range(NFC):
                        nc.sync.dma_start(
                            m_h[:, fi], m_in[fi * 128:(fi + 1) * 128, :])

                # v tiles whose 128-col s-window lies in this block
                for sv in range(kb * (KWB // 128), (kb + 1) * (KWB // 128)):
                    ps = psum_prep.tile([128, F], f32, tag="prep")
                    for fi in range(NFC):
                        nc.tensor.matmul(
                            ps[:],
                            xt_h[:, fi, sv * 128:(sv + 1) * 128],
                            wv_h[:, fi],
                            start=(fi == 0), stop=(fi == NFC - 1),
                        )
                    if sv % 2 == 0:
                        nc.vector.tensor_copy(v_h[:, sv], ps[:])
                    else:
                        nc.scalar.copy(v_h[:, sv], ps[:])

                if kb == 1:
                    # q-half columns 0..Q now resident: GT = (Xq @ M).T
                    for fo in range(NFC):
                        for qw in range(Q // 512):
                            ps = psum_prep.tile([128, 512], f32, tag="prep")
                            for fi in range(NFC):
                                nc.tensor.matmul(
                                    ps[:],
                                    m_h[:, fi, fo * 128:(fo + 1) * 128],
                                    xt_h[:, fi, qw * 512:(qw + 1) * 512],
                                    start=(fi == 0), stop=(fi == NFC - 1),
                                )
                            if qw % 2 == 0:
                                nc.vector.tensor_copy(
                                    gt_h[:, fo, qw * 512:(qw + 1) * 512], ps[:])
                            else:
                                nc.scalar.copy(
                                    gt_h[:, fo, qw * 512:(qw + 1) * 512], ps[:])

        # ---- main loop over q quarters ----
        with (
            tc.tile_pool(name="psum_s", bufs=3, space="PSUM") as psum_s,
            tc.tile_pool(name="psum_pv", bufs=4, space="PSUM") as psum_pv,
            tc.tile_pool(name="psum_z", bufs=1, space="PSUM") as psum_z,
            tc.tile_pool(name="expt", bufs=2) as expt_pool,
            tc.tile_pool(name="small", bufs=4) as small,
            tc.tile_pool(name="attn_stage", bufs=4) as attn_stage,
            tc.tile_pool(name="out_stage", bufs=2) as out_stage,
        ):
            for qq in range(NQW):
                qs = qq * QW
                expt = expt_pool.tile([128, NKC, QW], f16)

                # -- phase A: scores + exp + Z row accumulation --
                # zrow matmuls are emitted in batches of 4, at least 2
                # chunks behind the exp that produces their input, so the
                # in-order PE stream never waits on ACT and weight-set
                # switches (ones vs xt chunks) are amortized.
                zrow_ps = psum_z.tile([1, QW], f32)

                def _zrow(zc):
                    nc.tensor.matmul(
                        zrow_ps[:], ones_h[:], expt[:, zc],
                        start=(zc == 0), stop=(zc == NKC - 1),
                    )

                z_pending = []
                for c in range(NKC):
                    s_ps = psum_s.tile([128, QW], f32, tag="s")
                    for fi in range(NFC):
                        nc.tensor.matmul(
                            s_ps[:],
                            xt_h[:, fi, c * 128:(c + 1) * 128],
                            gt_h[:, fi, qs:qs + QW],
                            start=(fi == 0), stop=(fi == NFC - 1),
                        )
                    nc.scalar.activation(
                        expt[:, c], s_ps[:],
                        mybir.ActivationFunctionType.Exp,
                        bias=bias_c[:], scale=1.0,
                    )
                    z_pending.append(c)
                    if len(z_pending) >= 6:
                        for zc in z_pending[:4]:
                            _zrow(zc)
                        z_pending = z_pending[4:]
                for zc in z_pending:
                    _zrow(zc)

                # Z reciprocal + both orientations; overlaps phase B below.
                rz_row = small.tile([1, QW], f32, tag="rzrow")
                nc.vector.reciprocal(rz_row[:], zrow_ps[:])
                rz_bc = small.tile([128, QW], f32, tag="rzbc")
                nc.gpsimd.partition_broadcast(rz_bc[:], rz_row[:])
                rz_bh = small.tile([128, QW], f16, tag="rzbh")
                nc.vector.tensor_copy(rz_bh[:], rz_bc[:])
                # partition-oriented copy for the out normalization:
                # rzt[p, t] = rz_row[0, t*128+p] via a DRAM round-trip
                nc.sync.dma_start(rz_dram[qq:qq + 1, :], rz_row[:])
                rzt = small.tile([128, 4], f32, tag="rzt")
                nc.sync.dma_start(
                    rzt[:], rz_dram[qq].rearrange("(t p) -> p t", p=128))

                # -- attn.T writeout: expt * (1/Z); overlaps phase B --
                for c in range(NKC):
                    at = attn_stage.tile([128, QW], f32, tag="at")
                    nc.vector.tensor_mul(at[:], expt[:, c], rz_bh[:])
                    nc.sync.dma_start(
                        attn_t[c * 128:(c + 1) * 128, qs:qs + QW], at[:])

                # -- phase B: PV accumulation --
                pv_ps = [psum_pv.tile([128, F], f32, tag="pv", name=f"pv_{qq}_{t}")
                         for t in range(4)]
                for c in range(NKC):
                    for t in range(4):
                        nc.tensor.matmul(
                            pv_ps[t][:], expt[:, c, t * 128:(t + 1) * 128],
                            v_h[:, c],
                            start=(c == 0), stop=(c == NKC - 1),
                        )

                # -- out writeout: pv * (1/Z) (on DVE: keeps ACT's in-order
                # stream free for the next quarter's exp) --
                for t in range(4):
                    o_sb = out_stage.tile([128, F], f32, tag="o")
                    nc.vector.tensor_scalar_mul(o_sb[:], pv_ps[t][:],
                                                rzt[:, t:t + 1])
                    nc.sync.dma_start(
                        out[qs + t * 128:qs + (t + 1) * 128, :], o_sb[:])


def _comb(wr, wi):
    """torch complex nn.Linear y = x @ W.T folded to [x_r|x_i] @ Wc."""
    return np.block([[wr.T, wi.T], [-wi.T, wr.T]])


def _host_prep(inputs):
    f64 = np.float64
    wq = _comb(inputs["wq_real"].astype(f64), inputs["wq_imag"].astype(f64))
    wk = _comb(inputs["wk_real"].astype(f64), inputs["wk_imag"].astype(f64))
    wv = _comb(inputs["wv_real"].astype(f64), inputs["wv_imag"].astype(f64))
    m = ((wq @ wk.T) * SCALE).astype(np.float16)
    wv16 = wv.astype(np.float16)

    in_maps = []
    for c in range(8):
        b, h = c // 2, c % 2
        x = np.concatenate(
            [inputs["query_real"][b], inputs["query_imag"][b]], axis=-1)
        xt = np.ascontiguousarray(x.T.astype(np.float32))
        # roll q columns so this core's q-half occupies columns 0..Q
        if h == 1:
            xt = np.ascontiguousarray(np.roll(xt, -Q, axis=1))
        in_maps.append({"xt": xt, "m_in": m, "wv": wv16})
    return in_maps


_NC = None


def kernel(**inputs):
    global _NC
    if _NC is None:
        _NC = _build_nc()
    in_maps = _host_prep(inputs)
    res = run_bass_kernel_spmd(_NC, in_maps, list(range(8)))

    out_r = np.empty((B, S, D), np.float32)
    out_i = np.empty((B, S, D), np.float32)
    attn = np.empty((B, S, S), np.float32)
    for c in range(8):
        b, h = c // 2, c % 2
        qs = slice(h * Q, (h + 1) * Q)
        o = res.results[c]["out"]
        out_r[b, qs] = o[:, :D]
        out_i[b, qs] = o[:, D:]
        at = res.results[c]["attn_t"]  # [S(k rolled), Q]
        if h == 1:
            at = np.roll(at, Q, axis=0)
        attn[b, qs, :] = at.T
    return out_r, out_i, attn


# revision 34
# speedup vs baseline: 1.0839x; 1.0019x over previous
"""nn_ComplexAttention SPMD Bass kernel for 8 trn2 NeuronCores.

Math (per batch b):
  X = [x_r | x_i]                         [S, 2D]      (S=4096, D=256, 2D=F=512)
  q_flat = X @ Wq_c,  k_flat = X @ Wk_c,  v = X @ Wv_c  (complex linears folded
  into [F, F] block matrices built on the host)
  scores = q_flat @ k_flat.T * D**-0.5 = X @ M @ X.T,   M = Wq_c @ Wk_c.T * D**-0.5
  attn = softmax(scores, axis=-1);  out = [attn @ v_r | attn @ v_i]

Sharding: 8 cores = 4 batches x 2 q-halves (2048 q rows each). Each core
computes, entirely on-device in the [k, q] orientation (k on partitions so
softmax sums and attn @ v contract over partitions via the PE):
  HT = M @ X.T          [F, S]    (fp16, SBUF-resident)
  scoresT = HT.T @ X.T[:, q-half] per 512-col q-quarter, 128-row k-chunks
  expT = exp(scoresT - C) fp16    (C=8.5 keeps exp in fp16 range; cancels)
  Z (partition-sums of expT) via PE matmuls against a ones vector
  attn.T block = expT * (1/Z)     -> HBM  [S, 2048] f32
  out = (expT.T @ v) * (1/Z)      -> HBM  [2048, F] f32
Host side only slices/concats/transposes blocks to full-shape outputs.
"""

import numpy as np

import concourse.bass as bass
import concourse.mybir as mybir
import concourse.tile as tile
from concourse import bacc
from concourse.bass_utils import run_bass_kernel_spmd

B, S, D = 4, 4096, 256
F = 2 * D          # feature dim of the folded complex linear
Q = S // 2         # q rows per core
QW = 512           # q window (quarter of Q)
NQW = Q // QW      # 4 quarters
KC = 128           # k chunk (partition tile)
NKC = S // KC      # 32 k chunks
NFC = F // 128     # 4 feature chunks
EXP_SHIFT = 8.5    # exp(s - C): keeps exp within fp16 range; cancels in softmax
SCALE = D ** -0.5

f32 = mybir.dt.float32
f16 = mybir.dt.float16


def _build_nc():
    nc = bacc.Bacc("TRN2", target_bir_lowering=False, debug=False, num_devices=8)
    xt = nc.dram_tensor("xt", [F, S], f32, kind="ExternalInput").ap()
    m_in = nc.dram_tensor("m_in", [F, F], f16, kind="ExternalInput").ap()
    wv = nc.dram_tensor("wv", [F, F], f16, kind="ExternalInput").ap()
    attn_t = nc.dram_tensor("attn_t", [S, Q], f32, kind="ExternalOutput").ap()
    out = nc.dram_tensor("out", [Q, F], f32, kind="ExternalOutput").ap()
    rz_dram = nc.dram_tensor("rz_scratch", [NQW, QW], f32).ap()

    with tile.TileContext(nc) as tc:
        _emit(tc, xt, m_in, wv, attn_t, out, rz_dram)
    nc.compile()
    return nc


def _emit(tc, xt, m_in, wv, attn_t, out, rz_dram):
    # The host rolls xt's columns so this core's q-half occupies columns
    # 0..Q — one SPMD program serves both halves; attn.T rows come out in
    # the rolled k-order and are un-rolled on the host.
    nc = tc.nc
    KWB = 1024  # xt load/cast column-block

    with (
        tc.tile_pool(name="const", bufs=1) as const,
        tc.tile_pool(name="main", bufs=1) as main,
    ):
        # ---- constants ----
        ones_h = const.tile([128, 1], f16)
        nc.vector.memset(ones_h[:], 1.0)
        bias_c = const.tile([128, 1], f32)
        nc.vector.memset(bias_c[:], -EXP_SHIFT)

        m_h = const.tile([128, NFC, F], f16)
        wv_h = const.tile([128, NFC, F], f16)
        xt_h = main.tile([128, NFC, S], f16)
        gt_h = main.tile([128, NFC, Q], f16)
        v_h = main.tile([128, NKC, F], f16)

        with (
            tc.tile_pool(name="psum_prep", bufs=4, space="PSUM") as psum_prep,
            tc.tile_pool(name="stage", bufs=3) as stage_pool,
        ):
            # ---- load weights (already fp16 from the host); m is only
            # needed for GT at kb==1 so it loads after xt's first block ----
            for fi in range(NFC):
                nc.sync.dma_start(wv_h[:, fi], wv[fi * 128:(fi + 1) * 128, :])

            # ---- load + cast X.T in column blocks, computing v (and GT)
            # for each block as soon as its columns land ----
            for kb in range(S // KWB):
                cols = slice(kb * KWB, (kb + 1) * KWB)
                for fi in range(NFC):
                    st = stage_pool.tile([128, KWB], f32, tag="xstage")
                    nc.sync.dma_start(st[:], xt[fi * 128:(fi + 1) * 128, cols])
                    nc.vector.tensor_copy(xt_h[:, fi, cols], st[:])
                if kb == 0:
                    for fi in range(NFC):
                        nc.sync.dma_start(
                            m_h[:, fi], m_in[fi * 128:(fi + 1) * 128, :])

                # v tiles whose 128-col s-window lies in this block
                for sv in range(kb * (KWB // 128), (kb + 1) * (KWB // 128)):
                    ps = psum_prep.tile([128, F], f32, tag="prep")
                    for fi in range(NFC):
                        nc.tensor.matmul(
                            ps[:],
                            xt_h[:, fi, sv * 128:(sv + 1) * 128],
                            wv_h[:, fi],
                            start=(fi == 0), stop=(fi == NFC - 1),
                        )
                    if sv % 2 == 0:
                        nc.vector.tensor_copy(v_h[:, sv], ps[:])
                    else:
                        nc.scalar.copy(v_h[:, sv], ps[:])

                if kb == 1:
                    # q-half columns 0..Q now resident: GT = (Xq @ M).T
                    for fo in range(NFC):
                        for qw in range(Q // 512):
                            ps = psum_prep.tile([128, 512], f32, tag="prep")
                            for fi in range(NFC):
                                nc.tensor.matmul(
                                    ps[:],
                                    m_h[:, fi, fo * 128:(fo + 1) * 128],
                                    xt_h[:, fi, qw * 512:(qw + 1) * 512],
                                    start=(fi == 0), stop=(fi == NFC - 1),
                                )
                            if qw % 2 == 0:
                                nc.vector.tensor_copy(
                                    gt_h[:, fo, qw * 512:(qw + 1) * 512], ps[:])
                            else:
                                nc.scalar.copy(
                                    gt_h[:, fo, qw * 512:(qw + 1) * 512], ps[:])

        # ---- main loop over q quarters ----
        with (
            tc.tile_pool(name="psum_s", bufs=3, space="PSUM") as psum_s,
            tc.tile_pool(name="psum_pv", bufs=4, space="PSUM") as psum_pv,
            tc.tile_pool(name="psum_z", bufs=1, space="PSUM") as psum_z,
            tc.tile_pool(name="expt", bufs=2) as expt_pool,
            tc.tile_pool(name="small", bufs=4) as small,
            tc.tile_pool(name="attn_stage", bufs=4) as attn_stage,
            tc.tile_pool(name="out_stage", bufs=2) as out_stage,
        ):
            for qq in range(NQW):
                qs = qq * QW
                expt = expt_pool.tile([128, NKC, QW], f16)

                # -- phase A: scores + exp + Z row accumulation --
                # zrow matmuls are emitted in batches of 4, at least 2
                # chunks behind the exp that produces their input, so the
                # in-order PE stream never waits on ACT and weight-set
                # switches (ones vs xt chunks) are amortized.
                zrow_ps = psum_z.tile([1, QW], f32)

                def _zrow(zc):
                    nc.tensor.matmul(
                        zrow_ps[:], ones_h[:], expt[:, zc],
                        start=(zc == 0), stop=(zc == NKC - 1),
                    )

                z_pending = []
                for c in range(NKC):
                    s_ps = psum_s.tile([128, QW], f32, tag="s")
                    for fi in range(NFC):
                        nc.tensor.matmul(
                            s_ps[:],
                            xt_h[:, fi, c * 128:(c + 1) * 128],
                            gt_h[:, fi, qs:qs + QW],
                            start=(fi == 0), stop=(fi == NFC - 1),
                        )
                    nc.scalar.activation(
                        expt[:, c], s_ps[:],
                        mybir.ActivationFunctionType.Exp,
                        bias=bias_c[:], scale=1.0,
                    )
                    z_pending.append(c)
                    if len(z_pending) >= 6:
                        for zc in z_pending[:4]:
                            _zrow(zc)
                        z_pending = z_pending[4:]
                for zc in z_pending:
                    _zrow(zc)

                # Z reciprocal + both orientations; overlaps phase B below.
                rz_row = small.tile([1, QW], f32, tag="rzrow")
                nc.vector.reciprocal(rz_row[:], zrow_ps[:])
                rz_bc = small.tile([128, QW], f32, tag="rzbc")
                nc.gpsimd.partition_broadcast(rz_bc[:], rz_row[:])
                rz_bh = small.tile([128, QW], f16, tag="rzbh")
                nc.vector.tensor_copy(rz_bh[:], rz_bc[:])
                # partition-oriented copy for the out normalization:
                # rzt[p, t] = rz_row[0, t*128+p] via a DRAM round-trip
                nc.sync.dma_start(rz_dram[qq:qq + 1, :], rz_row[:])
                rzt = small.tile([128, 4], f32, tag="rzt")
                nc.sync.dma_start(
                    rzt[:], rz_dram[qq].rearrange("(t p) -> p t", p=128))

                # -- attn.T writeout: expt * (1/Z); overlaps phase B --
                for c in range(NKC):
                    at = attn_stage.tile([128, QW], f32, tag="at")
                    nc.vector.tensor_mul(at[:], expt[:, c], rz_bh[:])
                    nc.sync.dma_start(
                        attn_t[c * 128:(c + 1) * 128, qs:qs + QW], at[:])

                # -- phase B: PV accumulation --
                pv_ps = [psum_pv.tile([128, F], f32, tag="pv", name=f"pv_{qq}_{t}")
                         for t in range(4)]
                for c in range(NKC):
                    for t in range(4):
                        nc.tensor.matmul(
                            pv_ps[t][:], expt[:, c, t * 128:(t + 1) * 128],
                            v_h[:, c],
                            start=(c == 0), stop=(c == NKC - 1),
                        )

                # -- out writeout: pv * (1/Z) (on DVE: keeps ACT's in-order
                # stream free for the next quarter's exp) --
                for t in range(4):
                    o_sb = out_stage.tile([128, F], f32, tag="o")
                    nc.vector.tensor_scalar_mul(o_sb[:], pv_ps[t][:],
                                                rzt[:, t:t + 1])
                    nc.sync.dma_start(
                        out[qs + t * 128:qs + (t + 1) * 128, :], o_sb[:])


def _comb(wr, wi):
    """torch complex nn.Linear y = x @ W.T folded to [x_r|x_i] @ Wc."""
    return np.block([[wr.T, wi.T], [-wi.T, wr.T]])


def _host_prep(inputs):
    f64 = np.float64
    wq = _comb(inputs["wq_real"].astype(f64), inputs["wq_imag"].astype(f64))
    wk = _comb(inputs["wk_real"].astype(f64), inputs["wk_imag"].astype(f64))
    wv = _comb(inputs["wv_real"].astype(f64), inputs["wv_imag"].astype(f64))
    m = ((wq @ wk.T) * SCALE).astype(np.float16)
    wv16 = wv.astype(np.float16)

    in_maps = []
    for c in range(8):
        b, h = c // 2, c % 2
        x = np.concatenate(
            [inputs["query_real"][b], inputs["query_imag"][b]], axis=-1)
        xt = np.ascontiguousarray(x.T.astype(np.float32))
        # roll q columns so this core's q-half occupies columns 0..Q
        if h == 1:
            xt = np.ascontiguousarray(np.roll(xt, -Q, axis=1))
        in_maps.append({"xt": xt, "m_in": m, "wv": wv16})
    return in_maps


_NC = None


def kernel(**inputs):
    global _NC
    if _NC is None:
        _NC = _build_nc()
    in_maps = _host_prep(inputs)
    try:
        res = run_bass_kernel_spmd(_NC, in_maps, list(range(8)))
    except Exception:
        # transient NRT/device hiccups recover on retry
        res = run_bass_kernel_spmd(_NC, in_maps, list(range(8)))

    out_r = np.empty((B, S, D), np.float32)
    out_i = np.empty((B, S, D), np.float32)
    attn = np.empty((B, S, S), np.float32)
    for c in range(8):
        b, h = c // 2, c % 2
        qs = slice(h * Q, (h + 1) * Q)
        o = res.results[c]["out"]
        out_r[b, qs] = o[:, :D]
        out_i[b, qs] = o[:, D:]
        at = res.results[c]["attn_t"]  # [S(k rolled), Q]
        if h == 1:
            at = np.roll(at, Q, axis=0)
        attn[b, qs, :] = at.T
    return out_r, out_i, attn


# revision 35
# speedup vs baseline: 1.0858x; 1.0017x over previous
"""nn_ComplexAttention SPMD Bass kernel for 8 trn2 NeuronCores.

Math (per batch b):
  X = [x_r | x_i]                         [S, 2D]      (S=4096, D=256, 2D=F=512)
  q_flat = X @ Wq_c,  k_flat = X @ Wk_c,  v = X @ Wv_c  (complex linears folded
  into [F, F] block matrices built on the host)
  scores = q_flat @ k_flat.T * D**-0.5 = X @ M @ X.T,   M = Wq_c @ Wk_c.T * D**-0.5
  attn = softmax(scores, axis=-1);  out = [attn @ v_r | attn @ v_i]

Sharding: 8 cores = 4 batches x 2 q-halves (2048 q rows each). Each core
computes, entirely on-device in the [k, q] orientation (k on partitions so
softmax sums and attn @ v contract over partitions via the PE):
  GT = (X[q-half] @ M).T  [F, Q]  (fp16, SBUF-resident)
  v  = X @ Wv_c           [S, F]  (fp16, SBUF-resident)
  scoresT = X.T.T @ GT per 512-col q-quarter, 128-row k-chunks (xt stationary)
  expT = exp(scoresT - C) fp16    (C=8.5 keeps exp within fp16 range; the
                                   shift cancels in the softmax)
  Z (partition-sums of expT) via PE matmuls against a ones vector, emitted
  pipelined behind the exps; 1/Z derived in both orientations
  attn.T block = expT * (1/Z)     -> HBM  [S, 2048] f32
  out = (expT.T @ v) * (1/Z)      -> HBM  [2048, F] f32
Host side only slices/concats/transposes blocks to full-shape outputs.
"""

import numpy as np

import concourse.bass as bass
import concourse.mybir as mybir
import concourse.tile as tile
from concourse import bacc
from concourse.bass_utils import run_bass_kernel_spmd

B, S, D = 4, 4096, 256
F = 2 * D          # feature dim of the folded complex linear
Q = S // 2         # q rows per core
QW = 512           # q window (quarter of Q)
NQW = Q // QW      # 4 quarters
KC = 128           # k chunk (partition tile)
NKC = S // KC      # 32 k chunks
NFC = F // 128     # 4 feature chunks
EXP_SHIFT = 8.5    # exp(s - C): keeps exp within fp16 range; cancels in softmax
SCALE = D ** -0.5

f32 = mybir.dt.float32
f16 = mybir.dt.float16


def _build_nc():
    nc = bacc.Bacc("TRN2", target_bir_lowering=False, debug=False, num_devices=8)
    xt = nc.dram_tensor("xt", [F, S], f32, kind="ExternalInput").ap()
    m_in = nc.dram_tensor("m_in", [F, F], f16, kind="ExternalInput").ap()
    wv = nc.dram_tensor("wv", [F, F], f16, kind="ExternalInput").ap()
    attn_t = nc.dram_tensor("attn_t", [S, Q], f32, kind="ExternalOutput").ap()
    out = nc.dram_tensor("out", [Q, F], f32, kind="ExternalOutput").ap()
    rz_dram = nc.dram_tensor("rz_scratch", [NQW, QW], f32).ap()

    with tile.TileContext(nc) as tc:
        _emit(tc, xt, m_in, wv, attn_t, out, rz_dram)
    nc.compile()
    return nc


def _emit(tc, xt, m_in, wv, attn_t, out, rz_dram):
    # The host rolls xt's columns so this core's q-half occupies columns
    # 0..Q — one SPMD program serves both halves; attn.T rows come out in
    # the rolled k-order and are un-rolled on the host.
    nc = tc.nc
    KWB = 1024  # xt load/cast column-block

    with (
        tc.tile_pool(name="const", bufs=1) as const,
        tc.tile_pool(name="main", bufs=1) as main,
    ):
        # ---- constants ----
        ones_h = const.tile([128, 1], f16)
        nc.vector.memset(ones_h[:], 1.0)
        bias_c = const.tile([128, 1], f32)
        nc.vector.memset(bias_c[:], -EXP_SHIFT)

        m_h = const.tile([128, NFC, F], f16)
        wv_h = const.tile([128, NFC, F], f16)
        xt_h = main.tile([128, NFC, S], f16)
        gt_h = main.tile([128, NFC, Q], f16)
        v_h = main.tile([128, NKC, F], f16)

        with (
            tc.tile_pool(name="psum_prep", bufs=4, space="PSUM") as psum_prep,
            tc.tile_pool(name="stage", bufs=3) as stage_pool,
        ):
            # ---- load weights (already fp16 from the host); m is only
            # needed for GT at kb==1 so it loads after xt's first block ----
            for fi in range(NFC):
                nc.sync.dma_start(wv_h[:, fi], wv[fi * 128:(fi + 1) * 128, :])

            # ---- load + cast X.T in column blocks, computing v (and GT)
            # for each block as soon as its columns land ----
            for kb in range(S // KWB):
                cols = slice(kb * KWB, (kb + 1) * KWB)
                for fi in range(NFC):
                    st = stage_pool.tile([128, KWB], f32, tag="xstage")
                    nc.sync.dma_start(st[:], xt[fi * 128:(fi + 1) * 128, cols])
                    nc.vector.tensor_copy(xt_h[:, fi, cols], st[:])
                if kb == 0:
                    for fi in range(NFC):
                        nc.sync.dma_start(
                            m_h[:, fi], m_in[fi * 128:(fi + 1) * 128, :])

                # v tiles whose 128-col s-window lies in this block
                for sv in range(kb * (KWB // 128), (kb + 1) * (KWB // 128)):
                    ps = psum_prep.tile([128, F], f32, tag="prep")
                    for fi in range(NFC):
                        nc.tensor.matmul(
                            ps[:],
                            xt_h[:, fi, sv * 128:(sv + 1) * 128],
                            wv_h[:, fi],
                            start=(fi == 0), stop=(fi == NFC - 1),
                        )
                    if sv % 2 == 0:
                        nc.vector.tensor_copy(v_h[:, sv], ps[:])
                    else:
                        nc.scalar.copy(v_h[:, sv], ps[:])

                if kb == 1:
                    # q-half columns 0..Q now resident: GT = (Xq @ M).T
                    for fo in range(NFC):
                        for qw in range(Q // 512):
                            ps = psum_prep.tile([128, 512], f32, tag="prep")
                            for fi in range(NFC):
                                nc.tensor.matmul(
                                    ps[:],
                                    m_h[:, fi, fo * 128:(fo + 1) * 128],
                                    xt_h[:, fi, qw * 512:(qw + 1) * 512],
                                    start=(fi == 0), stop=(fi == NFC - 1),
                                )
                            if qw % 2 == 0:
                                nc.vector.tensor_copy(
                                    gt_h[:, fo, qw * 512:(qw + 1) * 512], ps[:])
                            else:
                                nc.scalar.copy(
                                    gt_h[:, fo, qw * 512:(qw + 1) * 512], ps[:])

        # ---- main loop over q quarters ----
        with (
            tc.tile_pool(name="psum_s", bufs=3, space="PSUM") as psum_s,
            tc.tile_pool(name="psum_pv", bufs=4, space="PSUM") as psum_pv,
            tc.tile_pool(name="psum_z", bufs=1, space="PSUM") as psum_z,
            tc.tile_pool(name="expt", bufs=2) as expt_pool,
            tc.tile_pool(name="small", bufs=4) as small,
            tc.tile_pool(name="attn_stage", bufs=4) as attn_stage,
            tc.tile_pool(name="out_stage", bufs=2) as out_stage,
        ):
            for qq in range(NQW):
                qs = qq * QW
                expt = expt_pool.tile([128, NKC, QW], f16)

                # -- phase A: scores + exp + Z row accumulation --
                # zrow matmuls are emitted in batches of 4, at least 2
                # chunks behind the exp that produces their input, so the
                # in-order PE stream never waits on ACT and weight-set
                # switches (ones vs xt chunks) are amortized.
                zrow_ps = psum_z.tile([1, QW], f32)

                def _zrow(zc):
                    nc.tensor.matmul(
                        zrow_ps[:], ones_h[:], expt[:, zc],
                        start=(zc == 0), stop=(zc == NKC - 1),
                    )

                z_pending = []
                for c in range(NKC):
                    s_ps = psum_s.tile([128, QW], f32, tag="s")
                    for fi in range(NFC):
                        nc.tensor.matmul(
                            s_ps[:],
                            xt_h[:, fi, c * 128:(c + 1) * 128],
                            gt_h[:, fi, qs:qs + QW],
                            start=(fi == 0), stop=(fi == NFC - 1),
                        )
                    nc.scalar.activation(
                        expt[:, c], s_ps[:],
                        mybir.ActivationFunctionType.Exp,
                        bias=bias_c[:], scale=1.0,
                    )
                    z_pending.append(c)
                    if len(z_pending) >= 6:
                        for zc in z_pending[:4]:
                            _zrow(zc)
                        z_pending = z_pending[4:]
                for zc in z_pending:
                    _zrow(zc)

                # Z reciprocal + both orientations; overlaps phase B below.
                rz_row = small.tile([1, QW], f32, tag="rzrow")
                nc.vector.reciprocal(rz_row[:], zrow_ps[:])
                rz_bc = small.tile([128, QW], f32, tag="rzbc")
                nc.gpsimd.partition_broadcast(rz_bc[:], rz_row[:])
                rz_bh = small.tile([128, QW], f16, tag="rzbh")
                nc.vector.tensor_copy(rz_bh[:], rz_bc[:])
                # partition-oriented copy for the out normalization:
                # rzt[p, t] = rz_row[0, t*128+p] via a DRAM round-trip
                nc.sync.dma_start(rz_dram[qq:qq + 1, :], rz_row[:])
                rzt = small.tile([128, 4], f32, tag="rzt")
                nc.sync.dma_start(
                    rzt[:], rz_dram[qq].rearrange("(t p) -> p t", p=128))

                # -- attn.T writeout: expt * (1/Z); overlaps phase B --
                for c in range(NKC):
                    at = attn_stage.tile([128, QW], f32, tag="at")
                    nc.vector.tensor_mul(at[:], expt[:, c], rz_bh[:])
                    nc.sync.dma_start(
                        attn_t[c * 128:(c + 1) * 128, qs:qs + QW], at[:])

                # -- phase B: PV accumulation --
                pv_ps = [psum_pv.tile([128, F], f32, tag="pv", name=f"pv_{qq}_{t}")
                         for t in range(4)]
                for c in range(NKC):
                    for t in range(4):
                        nc.tensor.matmul(
                            pv_ps[t][:], expt[:, c, t * 128:(t + 1) * 128],
                            v_h[:, c],
                            start=(c == 0), stop=(c == NKC - 1),
                        )

                # -- out writeout: pv * (1/Z) (on DVE: keeps ACT's in-order
                # stream free for the next quarter's exp) --
                for t in range(4):
                    o_sb = out_stage.tile([128, F], f32, tag="o")
                    nc.vector.tensor_scalar_mul(o_sb[:], pv_ps[t][:],
                                                rzt[:, t:t + 1])
                    nc.sync.dma_start(
                        out[qs + t * 128:qs + (t + 1) * 128, :], o_sb[:])


def _comb(wr, wi):
    """torch complex nn.Linear y = x @ W.T folded to [x_r|x_i] @ Wc."""
    return np.block([[wr.T, wi.T], [-wi.T, wr.T]])


def _host_prep(inputs):
    f64 = np.float64
    wq = _comb(inputs["wq_real"].astype(f64), inputs["wq_imag"].astype(f64))
    wk = _comb(inputs["wk_real"].astype(f64), inputs["wk_imag"].astype(f64))
    wv = _comb(inputs["wv_real"].astype(f64), inputs["wv_imag"].astype(f64))
    m = ((wq @ wk.T) * SCALE).astype(np.float16)
    wv16 = wv.astype(np.float16)

    in_maps = []
    for c in range(8):
        b, h = c // 2, c % 2
        x = np.concatenate(
            [inputs["query_real"][b], inputs["query_imag"][b]], axis=-1)
        xt = np.ascontiguousarray(x.T.astype(np.float32))
        # roll q columns so this core's q-half occupies columns 0..Q
        if h == 1:
            xt = np.ascontiguousarray(np.roll(xt, -Q, axis=1))
        in_maps.append({"xt": xt, "m_in": m, "wv": wv16})
    return in_maps


_NC = None


def kernel(**inputs):
    global _NC
    if _NC is None:
        _NC = _build_nc()
    in_maps = _host_prep(inputs)
    try:
        res = run_bass_kernel_spmd(_NC, in_maps, list(range(8)))
    except Exception:
        # transient NRT/device hiccups recover on retry
        res = run_bass_kernel_spmd(_NC, in_maps, list(range(8)))

    out_r = np.empty((B, S, D), np.float32)
    out_i = np.empty((B, S, D), np.float32)
    attn = np.empty((B, S, S), np.float32)
    for c in range(8):
        b, h = c // 2, c % 2
        qs = slice(h * Q, (h + 1) * Q)
        o = res.results[c]["out"]
        out_r[b, qs] = o[:, :D]
        out_i[b, qs] = o[:, D:]
        at = res.results[c]["attn_t"]  # [S(k rolled), Q]
        if h == 1:
            at = np.roll(at, Q, axis=0)
        attn[b, qs, :] = at.T
    return out_r, out_i, attn


# revision 37
# speedup vs baseline: 1.1059x; 1.0185x over previous
"""nn_ComplexAttention SPMD Bass kernel for 8 trn2 NeuronCores.

Math (per batch b):
  X = [x_r | x_i]                         [S, 2D]      (S=4096, D=256, 2D=F=512)
  q_flat = X @ Wq_c,  k_flat = X @ Wk_c,  v = X @ Wv_c  (complex linears folded
  into [F, F] block matrices built on the host)
  scores = q_flat @ k_flat.T * D**-0.5 = X @ M @ X.T,   M = Wq_c @ Wk_c.T * D**-0.5
  attn = softmax(scores, axis=-1);  out = [attn @ v_r | attn @ v_i]

Sharding: 8 cores = 4 batches x 2 q-halves (2048 q rows each). Each core
computes, entirely on-device in the [k, q] orientation (k on partitions so
softmax sums and attn @ v contract over partitions via the PE):
  GT = (X[q-half] @ M).T  [F, Q]  (fp16, SBUF-resident)
  v  = X @ Wv_c           [S, F]  (fp16, SBUF-resident)
  scoresT = X.T.T @ GT per 512-col q-quarter, 128-row k-chunks (xt stationary)
  expT = exp(scoresT - C) fp16    (C=8.5 keeps exp within fp16 range; the
                                   shift cancels in the softmax)
  Z (partition-sums of expT) via PE matmuls against a ones vector, emitted
  pipelined behind the exps; 1/Z derived in both orientations
  attn.T block = expT * (1/Z)     -> HBM  [S, 2048] f32
  out = (expT.T @ v) * (1/Z)      -> HBM  [2048, F] f32
Host side only slices/concats/transposes blocks to full-shape outputs.
"""

import numpy as np

import concourse.bass as bass
import concourse.mybir as mybir
import concourse.tile as tile
from concourse import bacc
from concourse.bass_utils import run_bass_kernel_spmd

B, S, D = 4, 4096, 256
F = 2 * D          # feature dim of the folded complex linear
Q = S // 2         # q rows per core
QW = 512           # q window (quarter of Q)
NQW = Q // QW      # 4 quarters
KC = 128           # k chunk (partition tile)
NKC = S // KC      # 32 k chunks
NFC = F // 128     # 4 feature chunks
EXP_SHIFT = 8.5    # exp(s - C): keeps exp within fp16 range; cancels in softmax
SCALE = D ** -0.5

f32 = mybir.dt.float32
f16 = mybir.dt.float16


def _build_nc():
    nc = bacc.Bacc("TRN2", target_bir_lowering=False, debug=False, num_devices=8)
    xt = nc.dram_tensor("xt", [F, S], f32, kind="ExternalInput").ap()
    m_in = nc.dram_tensor("m_in", [F, F], f16, kind="ExternalInput").ap()
    wv = nc.dram_tensor("wv", [F, F], f16, kind="ExternalInput").ap()
    attn_t = nc.dram_tensor("attn_t", [S, Q], f32, kind="ExternalOutput").ap()
    out = nc.dram_tensor("out", [Q, F], f32, kind="ExternalOutput").ap()
    rz_dram = nc.dram_tensor("rz_scratch", [NQW, QW], f32).ap()

    with tile.TileContext(nc) as tc:
        _emit(tc, xt, m_in, wv, attn_t, out, rz_dram)
    nc.compile()
    return nc


def _emit(tc, xt, m_in, wv, attn_t, out, rz_dram):
    # The host rolls xt's columns so this core's q-half occupies columns
    # 0..Q — one SPMD program serves both halves; attn.T rows come out in
    # the rolled k-order and are un-rolled on the host.
    nc = tc.nc
    KWB = 1024  # xt load/cast column-block

    with (
        tc.tile_pool(name="const", bufs=1) as const,
        tc.tile_pool(name="main", bufs=1) as main,
    ):
        # ---- constants ----
        ones_h = const.tile([128, 1], f16)
        nc.vector.memset(ones_h[:], 1.0)
        bias_c = const.tile([128, 1], f32)
        nc.vector.memset(bias_c[:], -EXP_SHIFT)

        m_h = const.tile([128, NFC, F], f16)
        wv_h = const.tile([128, NFC, F], f16)
        xt_h = main.tile([128, NFC, S], f16)
        gt_h = main.tile([128, NFC, Q], f16)
        v_h = main.tile([128, NKC, F], f16)

        with (
            tc.tile_pool(name="psum_prep", bufs=4, space="PSUM") as psum_prep,
            tc.tile_pool(name="stage", bufs=3) as stage_pool,
        ):
            # ---- load weights (already fp16 from the host); m is only
            # needed for GT at kb==1 so it loads after xt's first block ----
            for fi in range(NFC):
                nc.sync.dma_start(wv_h[:, fi], wv[fi * 128:(fi + 1) * 128, :])

            # ---- load + cast X.T in column blocks, computing v (and GT)
            # for each block as soon as its columns land ----
            for kb in range(S // KWB):
                cols = slice(kb * KWB, (kb + 1) * KWB)
                for fi in range(NFC):
                    st = stage_pool.tile([128, KWB], f32, tag="xstage")
                    nc.sync.dma_start(st[:], xt[fi * 128:(fi + 1) * 128, cols])
                    nc.vector.tensor_copy(xt_h[:, fi, cols], st[:])
                if kb == 0:
                    for fi in range(NFC):
                        nc.sync.dma_start(
                            m_h[:, fi], m_in[fi * 128:(fi + 1) * 128, :])

                # v tiles whose 128-col s-window lies in this block
                for sv in range(kb * (KWB // 128), (kb + 1) * (KWB // 128)):
                    ps = psum_prep.tile([128, F], f32, tag="prep")
                    for fi in range(NFC):
                        nc.tensor.matmul(
                            ps[:],
                            xt_h[:, fi, sv * 128:(sv + 1) * 128],
                            wv_h[:, fi],
                            start=(fi == 0), stop=(fi == NFC - 1),
                        )
                    if sv % 2 == 0:
                        nc.vector.tensor_copy(v_h[:, sv], ps[:])
                    else:
                        nc.scalar.copy(v_h[:, sv], ps[:])

                if kb == 1:
                    # q-half columns 0..Q now resident: GT = (Xq @ M).T
                    for fo in range(NFC):
                        for qw in range(Q // 512):
                            ps = psum_prep.tile([128, 512], f32, tag="prep")
                            for fi in range(NFC):
                                nc.tensor.matmul(
                                    ps[:],
                                    m_h[:, fi, fo * 128:(fo + 1) * 128],
                                    xt_h[:, fi, qw * 512:(qw + 1) * 512],
                                    start=(fi == 0), stop=(fi == NFC - 1),
                                )
                            if qw % 2 == 0:
                                nc.vector.tensor_copy(
                                    gt_h[:, fo, qw * 512:(qw + 1) * 512], ps[:])
                            else:
                                nc.scalar.copy(
                                    gt_h[:, fo, qw * 512:(qw + 1) * 512], ps[:])

        # ---- main loop over q quarters ----
        with (
            tc.tile_pool(name="psum_s", bufs=3, space="PSUM") as psum_s,
            tc.tile_pool(name="psum_pv", bufs=4, space="PSUM") as psum_pv,
            tc.tile_pool(name="psum_z", bufs=1, space="PSUM") as psum_z,
            tc.tile_pool(name="expt", bufs=2) as expt_pool,
            tc.tile_pool(name="small", bufs=4) as small,
            tc.tile_pool(name="attn_stage", bufs=4) as attn_stage,
            tc.tile_pool(name="out_stage", bufs=2) as out_stage,
        ):
            for qq in range(NQW):
                qs = qq * QW
                # Quarters 0-1 get Z from cheap N=1 minis inside phase B
                # (sharing the PV matmuls' already-loaded weights); their
                # normalize then drains against the TWO following quarters
                # of PE work. Quarters 2-3 accumulate Z during phase A
                # (zrow matmuls, batched 4-wide and >=2 chunks behind the
                # exp producing their input) so their normalize can start
                # before/during phase B — keeping the kernel tail short.
                early_z = qq >= 2
                expt = expt_pool.tile([128, NKC, QW], f16)

                # -- phase A: scores + exp (+ zrow for qq>=2) --
                if early_z:
                    zrow_ps = psum_z.tile([1, QW], f32, tag="z",
                                          name=f"zrow_{qq}")

                    def _zrow(zc):
                        nc.tensor.matmul(
                            zrow_ps[:], ones_h[:], expt[:, zc],
                            start=(zc == 0), stop=(zc == NKC - 1),
                        )

                z_pending = []
                for c in range(NKC):
                    s_ps = psum_s.tile([128, QW], f32, tag="s")
                    for fi in range(NFC):
                        nc.tensor.matmul(
                            s_ps[:],
                            xt_h[:, fi, c * 128:(c + 1) * 128],
                            gt_h[:, fi, qs:qs + QW],
                            start=(fi == 0), stop=(fi == NFC - 1),
                        )
                    nc.scalar.activation(
                        expt[:, c], s_ps[:],
                        mybir.ActivationFunctionType.Exp,
                        bias=bias_c[:], scale=1.0,
                    )
                    if early_z:
                        z_pending.append(c)
                        if len(z_pending) >= 6:
                            for zc in z_pending[:4]:
                                _zrow(zc)
                            z_pending = z_pending[4:]
                if early_z:
                    for zc in z_pending:
                        _zrow(zc)
                    # Z reciprocal + both orientations; overlaps phase B.
                    rz_row = small.tile([1, QW], f32, tag="rzrow")
                    nc.vector.reciprocal(rz_row[:], zrow_ps[:])
                    rz_bc = small.tile([128, QW], f32, tag="rzbc")
                    nc.gpsimd.partition_broadcast(rz_bc[:], rz_row[:])
                    rz_bh = small.tile([128, QW], f16, tag="rzbh")
                    nc.vector.tensor_copy(rz_bh[:], rz_bc[:])
                    nc.sync.dma_start(rz_dram[qq:qq + 1, :], rz_row[:])
                    rzt = small.tile([128, 4], f32, tag="rzt")
                    nc.sync.dma_start(
                        rzt[:], rz_dram[qq].rearrange("(t p) -> p t", p=128))

                    # attn.T writeout emitted here so it overlaps phase B
                    for c in range(NKC):
                        at = attn_stage.tile([128, QW], f32, tag="at")
                        nc.vector.tensor_mul(at[:], expt[:, c], rz_bh[:])
                        nc.sync.dma_start(
                            attn_t[c * 128:(c + 1) * 128, qs:qs + QW], at[:])

                # -- phase B: PV accumulation (+ Z minis for qq<2; all 4
                # columns form one accumulation group in one bank) --
                pv_ps = [psum_pv.tile([128, F], f32, tag="pv", name=f"pv_{qq}_{t}")
                         for t in range(4)]
                if not early_z:
                    zt_ps = psum_z.tile([128, 4], f32, tag="z",
                                        name=f"zt_{qq}")
                for c in range(NKC):
                    for t in range(4):
                        nc.tensor.matmul(
                            pv_ps[t][:], expt[:, c, t * 128:(t + 1) * 128],
                            v_h[:, c],
                            start=(c == 0), stop=(c == NKC - 1),
                        )
                        if not early_z:
                            nc.tensor.matmul(
                                zt_ps[:, t:t + 1],
                                expt[:, c, t * 128:(t + 1) * 128], ones_h[:],
                                start=(c == 0 and t == 0),
                                stop=(c == NKC - 1 and t == 3),
                            )

                if not early_z:
                    # rzt directly from the partition-oriented Z; the row
                    # orientation comes back via the DRAM round-trip
                    rzt = small.tile([128, 4], f32, tag="rzt")
                    nc.vector.reciprocal(rzt[:], zt_ps[:])
                    nc.sync.dma_start(
                        rz_dram[qq].rearrange("(t p) -> p t", p=128), rzt[:])
                    rz_bc = small.tile([128, QW], f32, tag="rzbc")
                    nc.sync.dma_start(
                        rz_bc[:],
                        rz_dram[qq:qq + 1, :].to_broadcast((128, QW)))
                    rz_bh = small.tile([128, QW], f16, tag="rzbh")
                    nc.vector.tensor_copy(rz_bh[:], rz_bc[:])

                    # attn.T writeout; drains against the next quarters
                    for c in range(NKC):
                        at = attn_stage.tile([128, QW], f32, tag="at")
                        nc.vector.tensor_mul(at[:], expt[:, c], rz_bh[:])
                        nc.sync.dma_start(
                            attn_t[c * 128:(c + 1) * 128, qs:qs + QW], at[:])

                # -- out writeout: pv * (1/Z) (on DVE: keeps ACT's in-order
                # stream free for the next quarter's exp) --
                for t in range(4):
                    o_sb = out_stage.tile([128, F], f32, tag="o")
                    nc.vector.tensor_scalar_mul(o_sb[:], pv_ps[t][:],
                                                rzt[:, t:t + 1])
                    nc.sync.dma_start(
                        out[qs + t * 128:qs + (t + 1) * 128, :], o_sb[:])


def _comb(wr, wi):
    """torch complex nn.Linear y = x @ W.T folded to [x_r|x_i] @ Wc."""
    return np.block([[wr.T, wi.T], [-wi.T, wr.T]])


def _host_prep(inputs):
    f64 = np.float64
    wq = _comb(inputs["wq_real"].astype(f64), inputs["wq_imag"].astype(f64))
    wk = _comb(inputs["wk_real"].astype(f64), inputs["wk_imag"].astype(f64))
    wv = _comb(inputs["wv_real"].astype(f64), inputs["wv_imag"].astype(f64))
    m = ((wq @ wk.T) * SCALE).astype(np.float16)
    wv16 = wv.astype(np.float16)

    in_maps = []
    for c in range(8):
        b, h = c // 2, c % 2
        x = np.concatenate(
            [inputs["query_real"][b], inputs["query_imag"][b]], axis=-1)
        xt = np.ascontiguousarray(x.T.astype(np.float32))
        # roll q columns so this core's q-half occupies columns 0..Q
        if h == 1:
            xt = np.ascontiguousarray(np.roll(xt, -Q, axis=1))
        in_maps.append({"xt": xt, "m_in": m, "wv": wv16})
    return in_maps


_NC = None


def kernel(**inputs):
    global _NC
    if _NC is None:
        _NC = _build_nc()
    in_maps = _host_prep(inputs)
    try:
        res = run_bass_kernel_spmd(_NC, in_maps, list(range(8)))
    except Exception:
        # transient NRT/device hiccups recover on retry
        res = run_bass_kernel_spmd(_NC, in_maps, list(range(8)))

    out_r = np.empty((B, S, D), np.float32)
    out_i = np.empty((B, S, D), np.float32)
    attn = np.empty((B, S, S), np.float32)
    for c in range(8):
        b, h = c // 2, c % 2
        qs = slice(h * Q, (h + 1) * Q)
        o = res.results[c]["out"]
        out_r[b, qs] = o[:, :D]
        out_i[b, qs] = o[:, D:]
        at = res.results[c]["attn_t"]  # [S(k rolled), Q]
        if h == 1:
            at = np.roll(at, Q, axis=0)
        attn[b, qs, :] = at.T
    return out_r, out_i, attn
